# revision 21
# baseline (speedup 1.0000x reference)
"""BranchLinear (MoE routing) Trainium2 kernel.

Math: out[t] = x[t] @ weight[branch_idx[t]] + bias[branch_idx[t]]
  x: [131072, 512] f32, branch_idx: [131072] int32 in [0,8),
  weight: [8, 512, 512] f32, bias: [8, 512] f32.

Strategy (data-parallel over 8 NeuronCores, T sharded by routing):
  Tokens are processed grouped by branch so each token is multiplied by
  exactly one 512x512 weight (1x FLOPs, vs 8x for the masked approach).
  The grouping permutation is cheap host-side bookkeeping (argsort of
  the given routing, applied while sharding): each branch's tokens are
  dealt round-robin across the 8 cores (so per-core branch counts are
  equal within 1, minimizing tile padding), each core's x shard is its
  16384 tokens in branch-sorted order (padded to 128-token tiles), and
  the core writes its output in that same sorted order; kernel() inverts
  the permutation while unsharding.

  With the shard pre-sorted, every device DMA is a plain sequential
  HWDGE transfer whose descriptors are generated by RTL at line rate.
  (Indirect SWDGE gathers/scatters were measured at ~17-24ns of GPSIMD
  Q7 descriptor-generation time per 2KB row — x2 for gather+scatter
  that's a ~600us/core floor, 3x the memory roofline. The HW consumes
  exactly one gather index per SBUF partition, so fat multi-tile
  indirect descriptors cannot express this access pattern either.)

  Per 128-token tile: load rows (SP HWDGE) -> PE transpose via identity
  matmul so D is on partitions (transpose data typed float32r: streams
  at 1.5 cycles/row on the PE, where plain fp32 runs at 2) -> 4
  accumulated bf16 matmuls (1 cycle/row) against the resident branch
  weight, kept in bf16 to halve the weight-load HBM traffic -> DVE adds
  the (pre-broadcast, f32) branch bias in the PSUM->SBUF move -> store
  rows (SP HWDGE). fp32 paths run at 1/4 matmul rate, and the 2e-2
  correctness gate leaves bf16 inputs ~5x margin.
"""

import numpy as np

P = 128           # SBUF partitions / tile height (tokens per tile)
NCORES = 8

_prog_cache = {}


def _split_multiwaits(nc):
    """This container's walrus build allows at most ONE sync wait per
    instruction (2 for EventSemaphore), but Tile emits instructions with
    several waits. Hoist extra waits onto fresh single-wait nops inserted
    just before the instruction on the same engine (identical blocking
    semantics: the engine's sequencer executes both in program order)."""
    import concourse.mybir as mybir

    uid = 0
    for f in nc.m.functions:
        for bb in f.blocks:
            insts = bb.instructions
            out, changed = [], False
            for ins in insts:
                si = ins.sync_info
                cap = 2 if ins.opcode == "EventSemaphore" else 1
                if si is not None and len(si.on_wait) > cap:
                    waits = list(si.on_wait)
                    for w in waits[cap:]:
                        nop = mybir.InstNoOp(
                            name=f"waitsplit_{uid}",
                            engine=ins.engine,
                            bass_nofuse=True,
                            sync_info=mybir.SyncInfo(on_wait=[w], on_update=[]),
                        )
                        uid += 1
                        nc.register_instruction(nop, overwrite=True)
                        out.append(nop)
                    si.on_wait = waits[:cap]
                    ins.sync_info = si
                    changed = True
                out.append(ins)
            if changed:
                bb.instructions = out
    return nc


def _build_program(SP_, D, NB, branch_of_tile, epochs=1):
    """Build the per-core SPMD bass program.

    Inputs (per core): xs [S*128, D] f32r (branch-sorted, padded shard),
    wr [NB*D, D] bf16 (weight reshaped), br [1, NB*D] f32.
    Output: out [S*128, D] f32 in the same sorted order.
    f32r is bit-identical to f32; it only switches the PE streaming mode.
    """
    import concourse.bass as bass
    import concourse.mybir as mybir
    import concourse.tile as tile
    from concourse.masks import make_identity

    f32 = mybir.dt.float32
    f32r = mybir.dt.float32r
    bf16 = mybir.dt.bfloat16
    KC = D // P                       # contraction chunks (4)
    S = len(branch_of_tile)           # total 128-token tiles
    assert SP_ == S * P

    nc = bass.Bass(name="branch_linear")
    x_d = nc.dram_tensor("xs", [S * P, D], f32r, kind="ExternalInput")
    w_d = nc.dram_tensor("wr", [NB * D, D], bf16, kind="ExternalInput")
    b_d = nc.dram_tensor("br", [1, NB * D], f32, kind="ExternalInput")
    out_d = nc.dram_tensor("out", [S * P, D], f32, kind="ExternalOutput")

    with tile.TileContext(nc) as tc:
        with (
            tc.tile_pool(name="const", bufs=1) as cpool,
            tc.tile_pool(name="xin", bufs=6) as gpool,
            tc.tile_pool(name="xt", bufs=4) as tpool,
            tc.tile_pool(name="osb", bufs=4) as opool,
            tc.tile_pool(name="ps_t", bufs=2, space="PSUM") as ps_t,
            tc.tile_pool(name="ps_o", bufs=2, space="PSUM") as ps_o,
            tc.tile_pool(name="ps_b", bufs=1, space="PSUM") as ps_b,
        ):
            # memset rejects f32r, so build the identity in f32 and convert
            # with a (bit-identical) copy
            ident_f32 = cpool.tile([P, P], f32, tag="ident_f32")
            make_identity(nc, ident_f32[:])
            ident = cpool.tile([P, P], f32r, tag="ident")
            nc.vector.tensor_copy(out=ident[:], in_=ident_f32[:])

            # resident weights: one [P, D] SBUF tile per (branch, k-chunk)
            w_sb = {}
            for n in range(NB):
                for k in range(KC):
                    w = cpool.tile([P, D], bf16, tag=f"w_{n}_{k}")
                    r0 = (n * KC + k) * P
                    nc.sync.dma_start(w[:], w_d[r0:r0 + P, :])
                    w_sb[(n, k)] = w

            # bias, broadcast to 128 partitions via K=1 matmul with ones
            bias1p = cpool.tile([1, NB * D], f32, tag="bias1p")
            nc.sync.dma_start(bias1p[:], b_d[:, :])
            ones1p = cpool.tile([1, P], f32, tag="ones1p")
            nc.vector.memset(ones1p[:], 1.0)
            bias_bc = cpool.tile([P, NB * D], f32, tag="bias_bc")
            for n in range(NB):
                pb = ps_b.tile([P, D], f32)
                nc.tensor.matmul(
                    out=pb[:], lhsT=ones1p[:], rhs=bias1p[:, n * D:(n + 1) * D],
                    start=True, stop=True,
                )
                nc.scalar.copy(out=bias_bc[:, n * D:(n + 1) * D], in_=pb[:])

            for s in list(range(S)) * epochs:
                n = branch_of_tile[s]
                # 1) sequential load of the tile's 128 sorted rows
                xg = gpool.tile([P, D], f32r, tag="xg")
                nc.sync.dma_start(xg[:], x_d[s * P:(s + 1) * P, :])
                # 2) transpose tile so D is on partitions (PE, identity)
                xt_ps = ps_t.tile([P, D], f32r)
                for k in range(KC):
                    nc.tensor.transpose(
                        out=xt_ps[:, k * P:(k + 1) * P],
                        in_=xg[:, k * P:(k + 1) * P],
                        identity=ident[:],
                    )
                xt = tpool.tile([P, D], bf16, tag="xt")
                nc.scalar.copy(out=xt[:], in_=xt_ps[:])
                # 3) out[tok, :] = sum_k xt[:,k].T @ W[n][k]
                o_ps = ps_o.tile([P, D], f32)
                for k in range(KC):
                    nc.tensor.matmul(
                        out=o_ps[:],
                        lhsT=xt[:, k * P:(k + 1) * P],
                        rhs=w_sb[(n, k)][:],
                        start=(k == 0), stop=(k == KC - 1),
                    )
                # 4) + bias (PSUM -> SBUF)
                o_sb = opool.tile([P, D], f32, tag="osb")
                nc.vector.tensor_add(
                    out=o_sb[:], in0=o_ps[:],
                    in1=bias_bc[:, n * D:(n + 1) * D],
                )
                # 5) sequential store to the same sorted slots
                nc.sync.dma_start(out_d[s * P:(s + 1) * P, :], o_sb[:])
    return _split_multiwaits(nc)


def _routing(branch_idx, TS, NB):
    """Branch-sorted, core-balanced shard bookkeeping.

    Each branch's tokens are dealt round-robin across cores (per-core
    counts equal within 1), then laid out per core grouped by branch and
    padded to 128-token tiles. Returns (gflat [NCORES][S*P] int64 GLOBAL
    token id per slot (pads dup token 0), valid [NCORES][S*P] bool,
    branch_of_tile [S])."""
    T = branch_idx.shape[0]
    ncores = T // TS
    order = np.argsort(branch_idx, kind="stable")   # tokens by branch
    counts = np.bincount(branch_idx, minlength=NB)
    slot_tiles = [int(-(-int(-(-counts[n] // ncores)) // P))
                  for n in range(NB)]
    branch_of_tile = []
    for n in range(NB):
        branch_of_tile += [n] * slot_tiles[n]
    S = len(branch_of_tile)

    gflat = np.zeros((ncores, S * P), np.int64)
    valid = np.zeros((ncores, S * P), bool)
    base = off = 0
    for n in range(NB):
        cnt = int(counts[n])
        toks = order[off:off + cnt]
        off += cnt
        for c in range(ncores):
            mine = toks[c::ncores]
            gflat[c, base:base + len(mine)] = mine
            valid[c, base:base + len(mine)] = True
        base += slot_tiles[n] * P
    return gflat, valid, branch_of_tile


def make_in_maps(x, wr, br, gflat):
    """Build per-core input maps; x is the FULL [T, D] f32 array and wr
    is the bf16-reshaped weight."""
    return [
        {"xs": np.ascontiguousarray(np.take(x, gflat[c], axis=0)),
         "wr": wr, "br": br}
        for c in range(NCORES)
    ]


def assemble_out(results, gflat, valid, T, D):
    out = np.empty((T, D), np.float32)
    for c in range(NCORES):
        out[gflat[c][valid[c]]] = results[c][valid[c]]
    return out


def kernel(x, branch_idx, weight, bias):
    import ml_dtypes
    from concourse.bass_utils import run_bass_kernel_spmd

    x = np.asarray(x, np.float32)
    branch_idx = np.asarray(branch_idx, np.int32)
    weight = np.asarray(weight, np.float32)
    bias = np.ascontiguousarray(np.asarray(bias, np.float32))

    T, D = x.shape
    NB = weight.shape[0]
    TS = T // NCORES

    gflat, valid, branch_of_tile = _routing(branch_idx, TS, NB)

    key = (TS, D, NB, tuple(branch_of_tile))
    if key not in _prog_cache:
        _prog_cache[key] = _build_program(
            len(branch_of_tile) * P, D, NB, branch_of_tile)
    nc = _prog_cache[key]

    wr = np.ascontiguousarray(
        weight.reshape(NB * D, D).astype(ml_dtypes.bfloat16))
    br = np.ascontiguousarray(bias.reshape(1, NB * D))
    in_maps = make_in_maps(x, wr, br, gflat)
    res = run_bass_kernel_spmd(nc, in_maps, core_ids=list(range(NCORES)))
    return assemble_out([res.results[c]["out"] for c in range(NCORES)],
                        gflat, valid, T, D)


# revision 23
# speedup vs baseline: 1.3248x; 1.3248x over previous
"""BranchLinear (MoE routing) Trainium2 kernel.

Math: out[t] = x[t] @ weight[branch_idx[t]] + bias[branch_idx[t]]
  x: [131072, 512] f32, branch_idx: [131072] int32 in [0,8),
  weight: [8, 512, 512] f32, bias: [8, 512] f32.

Strategy (data-parallel over 8 NeuronCores, T sharded by routing):
  Tokens are processed grouped by branch so each token is multiplied by
  exactly one 512x512 weight (1x FLOPs, vs 8x for the masked approach).
  The grouping permutation is cheap host-side bookkeeping (argsort of
  the given routing, applied while sharding): each branch's tokens are
  dealt round-robin across the 8 cores (so per-core branch counts are
  equal within 1, minimizing tile padding), each core's x shard is its
  16384 tokens in branch-sorted order (padded to 128-token tiles), and
  the core writes its output in that same sorted order; kernel() inverts
  the permutation while unsharding.

  With the shard pre-sorted, every device DMA is a plain sequential
  HWDGE transfer whose descriptors are generated by RTL at line rate.
  (Indirect SWDGE gathers/scatters were measured at ~17-24ns of GPSIMD
  Q7 descriptor-generation time per 2KB row — x2 for gather+scatter
  that's a ~600us/core floor, 3x the memory roofline. The HW consumes
  exactly one gather index per SBUF partition, so fat multi-tile
  indirect descriptors cannot express this access pattern either.)

  Per 128-token tile: load rows (SP HWDGE) -> PE transpose via identity
  matmul so D is on partitions (transpose data typed float32r: streams
  at 1.5 cycles/row on the PE, where plain fp32 runs at 2) -> 4
  accumulated bf16 matmuls (1 cycle/row) against the resident branch
  weight, kept in bf16 to halve the weight-load HBM traffic -> DVE adds
  the (pre-broadcast, f32) branch bias in the PSUM->SBUF move -> store
  rows (SP HWDGE). fp32 paths run at 1/4 matmul rate, and the 2e-2
  correctness gate leaves bf16 inputs ~5x margin.
"""

import numpy as np

P = 128           # SBUF partitions / tile height (tokens per tile)
NCORES = 8

_prog_cache = {}


def _split_multiwaits(nc):
    """This container's walrus build allows at most ONE sync wait per
    instruction (2 for EventSemaphore), but Tile emits instructions with
    several waits. Hoist extra waits onto fresh single-wait nops inserted
    just before the instruction on the same engine (identical blocking
    semantics: the engine's sequencer executes both in program order)."""
    import concourse.mybir as mybir

    uid = 0
    for f in nc.m.functions:
        for bb in f.blocks:
            insts = bb.instructions
            out, changed = [], False
            for ins in insts:
                si = ins.sync_info
                cap = 2 if ins.opcode == "EventSemaphore" else 1
                if si is not None and len(si.on_wait) > cap:
                    waits = list(si.on_wait)
                    for w in waits[cap:]:
                        nop = mybir.InstNoOp(
                            name=f"waitsplit_{uid}",
                            engine=ins.engine,
                            bass_nofuse=True,
                            sync_info=mybir.SyncInfo(on_wait=[w], on_update=[]),
                        )
                        uid += 1
                        nc.register_instruction(nop, overwrite=True)
                        out.append(nop)
                    si.on_wait = waits[:cap]
                    ins.sync_info = si
                    changed = True
                out.append(ins)
            if changed:
                bb.instructions = out
    return nc


def _build_program(SP_, D, NB, branch_of_tile, epochs=1):
    """Build the per-core SPMD bass program.

    Inputs (per core): xs [S*128, D] f32r (branch-sorted, padded shard),
    wr [NB*D, D] bf16 (weight reshaped), br [1, NB*D] f32.
    Output: out [S*128, D] f32 in the same sorted order.
    f32r is bit-identical to f32; it only switches the PE streaming mode.
    """
    import concourse.bass as bass
    import concourse.mybir as mybir
    import concourse.tile as tile
    from concourse.masks import make_identity

    f32 = mybir.dt.float32
    f32r = mybir.dt.float32r
    bf16 = mybir.dt.bfloat16
    KC = D // P                       # contraction chunks (4)
    S = len(branch_of_tile)           # total 128-token tiles
    assert SP_ == S * P

    nc = bass.Bass(name="branch_linear")
    x_d = nc.dram_tensor("xs", [S * P, D], f32r, kind="ExternalInput")
    w_d = nc.dram_tensor("wr", [NB * D, D], bf16, kind="ExternalInput")
    b_d = nc.dram_tensor("br", [1, NB * D], f32, kind="ExternalInput")
    out_d = nc.dram_tensor("out", [S * P, D], f32, kind="ExternalOutput")

    with tile.TileContext(nc) as tc:
        with (
            tc.tile_pool(name="const", bufs=1) as cpool,
            tc.tile_pool(name="xin", bufs=6) as gpool,
            tc.tile_pool(name="xt", bufs=4) as tpool,
            tc.tile_pool(name="osb", bufs=4) as opool,
            tc.tile_pool(name="ps_t", bufs=2, space="PSUM") as ps_t,
            tc.tile_pool(name="ps_o", bufs=2, space="PSUM") as ps_o,
            tc.tile_pool(name="ps_b", bufs=1, space="PSUM") as ps_b,
        ):
            # memset rejects f32r, so build the identity in f32 and convert
            # with a (bit-identical) copy
            ident_f32 = cpool.tile([P, P], f32, tag="ident_f32")
            make_identity(nc, ident_f32[:])
            ident = cpool.tile([P, P], f32r, tag="ident")
            nc.vector.tensor_copy(out=ident[:], in_=ident_f32[:])

            # resident weights: one [P, D] SBUF tile per (branch, k-chunk)
            w_sb = {}
            for n in range(NB):
                for k in range(KC):
                    w = cpool.tile([P, D], bf16, tag=f"w_{n}_{k}")
                    r0 = (n * KC + k) * P
                    nc.sync.dma_start(w[:], w_d[r0:r0 + P, :])
                    w_sb[(n, k)] = w

            # bias, broadcast to 128 partitions via K=1 matmul with ones
            bias1p = cpool.tile([1, NB * D], f32, tag="bias1p")
            nc.sync.dma_start(bias1p[:], b_d[:, :])
            ones1p = cpool.tile([1, P], f32, tag="ones1p")
            nc.vector.memset(ones1p[:], 1.0)
            bias_bc = cpool.tile([P, NB * D], f32, tag="bias_bc")
            for n in range(NB):
                pb = ps_b.tile([P, D], f32)
                nc.tensor.matmul(
                    out=pb[:], lhsT=ones1p[:], rhs=bias1p[:, n * D:(n + 1) * D],
                    start=True, stop=True,
                )
                nc.scalar.copy(out=bias_bc[:, n * D:(n + 1) * D], in_=pb[:])

            # 4 tiles per DMA instruction: dest [128, 4, D] with source
            # rows g*128+p — one SEQ config + one HWDGE setup per 4 tiles
            # instead of per tile, same 2KB-per-partition-run descriptors.
            G = 4
            xv = x_d[:, :].rearrange("(s g p) d -> s p g d", g=G, p=P)
            ov = out_d[:, :].rearrange("(s g p) d -> s p g d", g=G, p=P)
            for s0 in list(range(0, S, G)) * epochs:
                # 1) sequential load of 4 tiles' sorted rows
                xg = gpool.tile([P, G, D], f32r, tag="xg")
                nc.sync.dma_start(xg[:], xv[s0 // G])
                o_sb = opool.tile([P, G, D], f32, tag="osb")
                for g in range(G):
                    n = branch_of_tile[s0 + g]
                    # 2) transpose tile so D is on partitions (PE, identity)
                    xt_ps = ps_t.tile([P, D], f32r)
                    for k in range(KC):
                        nc.tensor.transpose(
                            out=xt_ps[:, k * P:(k + 1) * P],
                            in_=xg[:, g, k * P:(k + 1) * P],
                            identity=ident[:],
                        )
                    xt = tpool.tile([P, D], bf16, tag="xt")
                    nc.scalar.copy(out=xt[:], in_=xt_ps[:])
                    # 3) out[tok, :] = sum_k xt[:,k].T @ W[n][k]
                    o_ps = ps_o.tile([P, D], f32)
                    for k in range(KC):
                        nc.tensor.matmul(
                            out=o_ps[:],
                            lhsT=xt[:, k * P:(k + 1) * P],
                            rhs=w_sb[(n, k)][:],
                            start=(k == 0), stop=(k == KC - 1),
                        )
                    # 4) + bias (PSUM -> SBUF)
                    nc.vector.tensor_add(
                        out=o_sb[:, g, :], in0=o_ps[:],
                        in1=bias_bc[:, n * D:(n + 1) * D],
                    )
                # 5) sequential store to the same sorted slots
                nc.sync.dma_start(ov[s0 // G], o_sb[:])
    return _split_multiwaits(nc)


def _routing(branch_idx, TS, NB):
    """Branch-sorted, core-balanced shard bookkeeping.

    Each branch's tokens are dealt round-robin across cores (per-core
    counts equal within 1), then laid out per core grouped by branch and
    padded to 128-token tiles. Returns (gflat [NCORES][S*P] int64 GLOBAL
    token id per slot (pads dup token 0), valid [NCORES][S*P] bool,
    branch_of_tile [S])."""
    T = branch_idx.shape[0]
    ncores = T // TS
    order = np.argsort(branch_idx, kind="stable")   # tokens by branch
    counts = np.bincount(branch_idx, minlength=NB)
    slot_tiles = [int(-(-int(-(-counts[n] // ncores)) // P))
                  for n in range(NB)]
    branch_of_tile = []
    for n in range(NB):
        branch_of_tile += [n] * slot_tiles[n]
    while len(branch_of_tile) % 4:       # whole 4-tile DMA groups
        branch_of_tile.append(NB - 1)
    S = len(branch_of_tile)

    gflat = np.zeros((ncores, S * P), np.int64)
    valid = np.zeros((ncores, S * P), bool)
    base = off = 0
    for n in range(NB):
        cnt = int(counts[n])
        toks = order[off:off + cnt]
        off += cnt
        for c in range(ncores):
            mine = toks[c::ncores]
            gflat[c, base:base + len(mine)] = mine
            valid[c, base:base + len(mine)] = True
        base += slot_tiles[n] * P
    return gflat, valid, branch_of_tile


def make_in_maps(x, wr, br, gflat):
    """Build per-core input maps; x is the FULL [T, D] f32 array and wr
    is the bf16-reshaped weight."""
    return [
        {"xs": np.ascontiguousarray(np.take(x, gflat[c], axis=0)),
         "wr": wr, "br": br}
        for c in range(NCORES)
    ]


def assemble_out(results, gflat, valid, T, D):
    out = np.empty((T, D), np.float32)
    for c in range(NCORES):
        out[gflat[c][valid[c]]] = results[c][valid[c]]
    return out


def kernel(x, branch_idx, weight, bias):
    import ml_dtypes
    from concourse.bass_utils import run_bass_kernel_spmd

    x = np.asarray(x, np.float32)
    branch_idx = np.asarray(branch_idx, np.int32)
    weight = np.asarray(weight, np.float32)
    bias = np.ascontiguousarray(np.asarray(bias, np.float32))

    T, D = x.shape
    NB = weight.shape[0]
    TS = T // NCORES

    gflat, valid, branch_of_tile = _routing(branch_idx, TS, NB)

    key = (TS, D, NB, tuple(branch_of_tile))
    if key not in _prog_cache:
        _prog_cache[key] = _build_program(
            len(branch_of_tile) * P, D, NB, branch_of_tile)
    nc = _prog_cache[key]

    wr = np.ascontiguousarray(
        weight.reshape(NB * D, D).astype(ml_dtypes.bfloat16))
    br = np.ascontiguousarray(bias.reshape(1, NB * D))
    in_maps = make_in_maps(x, wr, br, gflat)
    res = run_bass_kernel_spmd(nc, in_maps, core_ids=list(range(NCORES)))
    return assemble_out([res.results[c]["out"] for c in range(NCORES)],
                        gflat, valid, T, D)
